# revision 31
# baseline (speedup 1.0000x reference)
"""Mistral4-style MoE block on 8 Trainium2 NeuronCores — single merged program.

Strategy (expert-parallel, sparse compute):
  - Router (sigmoid gate + top-4, weight normalization) runs on host in
    float64: tiny compute, gives the exact token->expert dispatch lists.
  - Host pre-gathers each expert's token batch and pre-tiles ALL weights
    into the exact SBUF layouts the matmuls need (contiguous 1-2MB blocks,
    no on-device DMA transposes). Shared-expert weights ship replicated
    (no device AllGather: collectives contend with the weight-stream DMAs
    for the 16 SDMA engines).
  - Experts are paired big+small per core (sorted by token count), so slot
    capacities are C0+C1 ~= max+median instead of 2*max.
  - One program per core:
      * slot0/slot1 (routed): gated MLP on the pre-gathered token batch;
        the per-token combine weight is folded into the PSUM-drain copy's
        `scale` operand; outputs scatter-add (indirect DMA, bf16) into
        column-half partial tensors [T, 2048].
      * two ReduceScatters (one per half) fire after all routed
        scatter-adds COMPLETE (collectives freeze concurrent DMA drain on
        this stack, so they must never overlap in-flight scatters); they
        are spaced apart so the shared slot's weight-prefetch buffers can
        coast through each window.
      * shared expert (slot2, tokens [512c,512(c+1))) runs last,
        overlapping the ReduceScatters; its output stays out of the RS.
      * final[512,H] = rs + shared_y.
"""

import sys

if "/opt/trn_rl_repo" not in sys.path:
    sys.path.insert(0, "/opt/trn_rl_repo")

import numpy as np
import ml_dtypes

T, H, I, E, TOPK = 4096, 4096, 2048, 16, 4
N_CORES = 8
CS_SHARED = T // N_CORES  # 512 shared-expert tokens per core
HK = H // 128  # 32 contraction chunks for up/gate
IK = I // 128  # 16 contraction chunks for down-proj
BF16 = ml_dtypes.bfloat16

_cache = {}


def _csplits(c, step=512):
    return [(c0, min(step, c - c0)) for c0 in range(0, c, step)]


# --------------------------------------------------------------------------
# program builder
# --------------------------------------------------------------------------

def _build_moe(C0, C1, wgu_bufs=3, wd_bufs=2, psa_bufs=4, psb_bufs=4,
               stage_bufs=2, ot_bufs=6, xt_chunk=512, nrs=2,
               memset_gate_i=6, rs_gate_i=8):
    import concourse.mybir as mybir
    import concourse.tile as tile
    import concourse.bass as bass
    from concourse.tile import add_dep_helper
    from concourse import bacc

    nc = bacc.Bacc("TRN2", target_bir_lowering=False, debug=False)
    dt = mybir.dt

    QW = H // nrs            # quarter width (1024 for nrs=4)
    HPQ = QW // 512          # h-blocks per quarter

    # ---- per-core external inputs ----
    xt0_d = nc.dram_tensor("xt0", [128, HK, C0], dt.bfloat16, kind="ExternalInput")
    xt1_d = nc.dram_tensor("xt1", [128, HK, C1], dt.bfloat16, kind="ExternalInput")
    xts_d = nc.dram_tensor("xts", [128, HK, CS_SHARED], dt.bfloat16,
                           kind="ExternalInput")
    # up/gate weights, tiled: [slot, i_block, p, (gu, k, c)]
    wgu_d = nc.dram_tensor("wgu", [2, IK, 128, 2 * HK * 128], dt.bfloat16,
                           kind="ExternalInput")
    # down weights, tiled: [slot, h_block, p, (k, n)]
    wd_d = nc.dram_tensor("wd", [2, 8, 128, IK * 512], dt.bfloat16,
                          kind="ExternalInput")
    # shared-expert weights (full, replicated to every core)
    sgu_d = nc.dram_tensor("sgu", [IK, 128, 2 * HK * 128], dt.bfloat16,
                           kind="ExternalInput")
    sd_d = nc.dram_tensor("sd", [8, 128, IK * 512], dt.bfloat16,
                          kind="ExternalInput")
    idx0_d = nc.dram_tensor("idx0", [128, C0 // 128], dt.int32,
                            kind="ExternalInput")
    idx1_d = nc.dram_tensor("idx1", [128, C1 // 128], dt.int32,
                            kind="ExternalInput")
    w0_d = nc.dram_tensor("w0", [128, C0 // 128], dt.float32,
                          kind="ExternalInput")
    w1_d = nc.dram_tensor("w1", [128, C1 // 128], dt.float32,
                          kind="ExternalInput")
    final_d = nc.dram_tensor("final", [CS_SHARED, H], dt.float32,
                             kind="ExternalOutput")

    # ---- internal DRAM ----
    partial_q = [nc.dram_tensor(f"partial{q}", [T, QW], dt.bfloat16)
                 for q in range(nrs)]
    rs_q = [nc.dram_tensor(f"rs{q}", [CS_SHARED, QW], dt.bfloat16)
            for q in range(nrs)]
    ys_dram = nc.dram_tensor("ys", [CS_SHARED, H], dt.bfloat16)

    with tile.TileContext(nc) as tc:
        with (
            tc.tile_pool(name="xT", bufs=1) as xT_pool,
            tc.tile_pool(name="zT", bufs=1) as zT_pool,
            tc.tile_pool(name="wgu", bufs=wgu_bufs) as wgu_pool,
            tc.tile_pool(name="wd", bufs=wd_bufs) as wd_pool,
            tc.tile_pool(name="stage", bufs=stage_bufs) as stage_pool,
            tc.tile_pool(name="ot", bufs=ot_bufs) as ot_pool,
            tc.tile_pool(name="wcol", bufs=2) as wcol_pool,
            tc.tile_pool(name="idx", bufs=2) as idx_pool,
            tc.tile_pool(name="zero", bufs=1) as zero_pool,
            tc.tile_pool(name="fin", bufs=2) as fin_pool,
            tc.tile_pool(name="psA", bufs=psa_bufs, space="PSUM") as psum_a,
            tc.tile_pool(name="psB", bufs=psb_bufs, space="PSUM") as psum_b,
        ):
            slots = [
                (xt0_d, C0, 0, idx0_d, w0_d),
                (xt1_d, C1, 1, idx1_d, w1_d),
                (xts_d, CS_SHARED, None, None, None),
            ]

            rs_insts = []
            for s, (xt_ap, Cs, wslot, idx_ap, w_ap) in enumerate(slots):
                ct = Cs // 128

                # load xT in k-plane chunks: contiguous per-partition runs
                # (one descriptor per partition) instead of 1KB fragments.
                # The first chunk is small so its completion (which gates the
                # very first matmul) lands early even though queued chunk DMAs
                # drain round-robin.
                xT = xT_pool.tile([128, HK, Cs], dt.bfloat16, tag="xT")
                for k0, k1 in ((0, 4), (4, 12), (12, 22), (22, HK)):
                    nc.scalar.dma_start(out=xT[:, k0:k1, :],
                                        in_=xt_ap[:, k0:k1, :])

                if idx_ap is not None:
                    it = idx_pool.tile([128, ct], dt.int32, tag="idx")
                    nc.sync.dma_start(out=it[:], in_=idx_ap[:, :ct])
                    wc = wcol_pool.tile([128, ct], dt.float32, tag="wcol")
                    nc.sync.dma_start(out=wc[:], in_=w_ap[:, :ct])

                # ---- up/gate -> zT ----
                zT = zT_pool.tile([128, IK, Cs], dt.bfloat16, tag="zT")
                for i in range(IK):
                    wgu_i = wgu_pool.tile([128, 2, HK, 128], dt.bfloat16,
                                          tag="wgu")
                    src = wgu_d[wslot, i] if wslot is not None else sgu_d[i]
                    wgu_load = nc.sync.dma_start(
                        out=wgu_i[:],
                        in_=src.rearrange("p (g k c) -> p g k c", g=2, k=HK),
                    )
                    # space the second ReduceScatter into mid-shared-slot so
                    # the weight-prefetch buffers can coast through each RS
                    # window separately
                    if s == 2 and i == rs_gate_i and len(rs_insts) < nrs:
                        q = len(rs_insts)
                        cc = nc.gpsimd.collective_compute(
                            "ReduceScatter", mybir.AluOpType.add,
                            replica_groups=[list(range(N_CORES))],
                            ins=[partial_q[q][:]], outs=[rs_q[q][:]],
                        )
                        add_dep_helper(cc.ins, wgu_load.ins,
                                       reason="space RS into shared slot")
                        rs_insts.append(cc)
                    for c0, cw in _csplits(Cs):
                        pg = psum_a.tile([128, cw], dt.float32, tag="psA")
                        pu = psum_a.tile([128, cw], dt.float32, tag="psA")
                        for k in range(HK):
                            nc.tensor.matmul(
                                pg[:], wgu_i[:, 0, k, :], xT[:, k, c0:c0 + cw],
                                start=(k == 0), stop=(k == HK - 1),
                            )
                        for k in range(HK):
                            nc.tensor.matmul(
                                pu[:], wgu_i[:, 1, k, :], xT[:, k, c0:c0 + cw],
                                start=(k == 0), stop=(k == HK - 1),
                            )
                        g_s = stage_pool.tile([128, cw], dt.float32, tag="stage")
                        nc.scalar.activation(
                            g_s[:], pg[:], mybir.ActivationFunctionType.Silu
                        )
                        nc.vector.tensor_mul(zT[:, i, c0:c0 + cw], g_s[:], pu[:])

                    # zero-init the partial quarters off the critical head,
                    # hidden under slot0's up/gate
                    if s == 0 and i == memset_gate_i:
                        # hold the memset off the critical head: the idle
                        # gpsimd queue would otherwise flood HBM at t=0 and
                        # starve the weight/xt loads the PE waits on. The
                        # 1-element rewrite below adds a real data dep on this
                        # i-iteration's silu output, so the stores can't be
                        # scheduled before mid-slot0.
                        zb = 4096 // QW if QW < 4096 else 1
                        zt = zero_pool.tile([128, QW], dt.bfloat16)
                        nc.vector.memset(zt[:], 0.0)
                        nc.vector.tensor_scalar_mul(
                            zt[0:1, 0:1], g_s[0:1, 0:1], 0.0)
                        for q in range(nrs):
                            for tb in range(T // 128):
                                nc.gpsimd.dma_start(
                                    out=partial_q[q][tb * 128:(tb + 1) * 128,
                                                     :],
                                    in_=zt[:])

                # ---- down-proj -> scatter-add / ys ----
                for h in range(8):
                    wd_h = wd_pool.tile([128, IK, 512], dt.bfloat16, tag="wd")
                    src = wd_d[wslot, h] if wslot is not None else sd_d[h]
                    nc.sync.dma_start(
                        out=wd_h[:],
                        in_=src.rearrange("p (k n) -> p k n", k=IK),
                    )
                    for cb in range(ct):
                        po = psum_b.tile([128, 512], dt.float32, tag="psB")
                        for k in range(IK):
                            nc.tensor.matmul(
                                po[:], zT[:, k, cb * 128:(cb + 1) * 128],
                                wd_h[:, k, :],
                                start=(k == 0), stop=(k == IK - 1),
                            )
                        ot = ot_pool.tile([128, 512], dt.bfloat16, tag="ot")
                        if idx_ap is not None:
                            nc.scalar.activation(
                                ot[:], po[:], mybir.ActivationFunctionType.Copy,
                                scale=wc[:, cb:cb + 1],
                            )
                            nc.gpsimd.indirect_dma_start(
                                out=partial_q[h // HPQ][:],
                                out_offset=bass.IndirectOffsetOnAxis(
                                    ap=it[:, cb:cb + 1], axis=0),
                                in_=ot[:],
                                in_offset=None,
                                element_offset=(h % HPQ) * 512,
                                compute_op=mybir.AluOpType.add,
                            )
                        else:
                            nc.scalar.activation(
                                ot[:], po[:], mybir.ActivationFunctionType.Copy
                            )
                            nc.scalar.dma_start(
                                out=ys_dram[cb * 128:(cb + 1) * 128,
                                            h * 512:(h + 1) * 512],
                                in_=ot[:],
                            )

                # first ReduceScatter fires once all routed scatter-adds have
                # COMPLETED (a collective freezes concurrent DMA drain, so an
                # RS that starts while scatters are still in flight deadlocks
                # the drain pipeline until the RS finishes). The probe reads
                # force completion-ordering: their dispatch waits on every
                # partial writer's DMA-completion semaphore, and the RS sits
                # behind them on the in-order gpsimd queue.
                if s == 1:
                    for q in range(nrs):
                        probe = fin_pool.tile([128, 16], dt.bfloat16,
                                              tag="rsb")
                        nc.gpsimd.dma_start(out=probe[:],
                                            in_=partial_q[q][0:128, 0:16])
                    cc = nc.gpsimd.collective_compute(
                        "ReduceScatter", mybir.AluOpType.add,
                        replica_groups=[list(range(N_CORES))],
                        ins=[partial_q[0][:]], outs=[rs_q[0][:]],
                    )
                    rs_insts.append(cc)

            # ---- final = rs + ys ----
            # q-major: the q=0 half only needs RS_lo + the shared expert's
            # h<4 down-proj outputs, so it completes while RS_hi / the rest
            # of the shared slot still run. rs loads ride the sync ring (the
            # gpsimd queue would serialize them behind the second RS).
            FW = 256
            for f0 in range(0, H, FW):
                q, qo = f0 // QW, f0 % QW
                for rb in range(CS_SHARED // 128):
                    rsb = fin_pool.tile([128, FW], dt.bfloat16, tag="rsb")
                    nc.sync.dma_start(
                        out=rsb[:],
                        in_=rs_q[q][rb * 128:(rb + 1) * 128, qo:qo + FW])
                    ysb = fin_pool.tile([128, FW], dt.bfloat16, tag="ysb")
                    nc.scalar.dma_start(
                        out=ysb[:],
                        in_=ys_dram[rb * 128:(rb + 1) * 128, f0:f0 + FW])
                    ob = fin_pool.tile([128, FW], dt.float32, tag="ob")
                    nc.vector.tensor_add(ob[:], rsb[:], ysb[:])
                    nc.sync.dma_start(
                        out=final_d[rb * 128:(rb + 1) * 128, f0:f0 + FW],
                        in_=ob[:])

    nc.compile()
    return nc


# --------------------------------------------------------------------------
# execution plumbing (cached jitted SPMD launch)
# --------------------------------------------------------------------------

def _mesh_shard():
    import jax
    from jax.sharding import Mesh, PartitionSpec, NamedSharding

    if "mesh" not in _cache:
        devices = jax.devices()[:N_CORES]
        mesh = Mesh(np.asarray(devices), ("core",))
        _cache["mesh"] = mesh
        _cache["shard"] = NamedSharding(mesh, PartitionSpec("core"))
    return _cache["mesh"], _cache["shard"]


def _exec_handle(nc):
    import jax
    import jax.numpy as jnp
    from jax.sharding import PartitionSpec
    from jax.experimental.shard_map import shard_map
    import concourse.mybir as mybir
    from concourse import bass2jax

    key = id(nc)
    if key in _cache:
        return _cache[key]

    bass2jax.install_neuronx_cc_hook()
    mesh, shard = _mesh_shard()

    part_name = nc.partition_id_tensor.name if nc.partition_id_tensor else None
    in_names, out_names, out_avals = [], [], []
    for alloc in nc.m.functions[0].allocations:
        if not isinstance(alloc, mybir.MemoryLocationSet):
            continue
        name = alloc.memorylocations[0].name
        if alloc.kind == "ExternalInput":
            if name != part_name:
                in_names.append(name)
        elif alloc.kind == "ExternalOutput":
            out_names.append(name)
            out_avals.append(
                jax.core.ShapedArray(tuple(alloc.tensor_shape),
                                     mybir.dt.np(alloc.dtype))
            )
    n_params = len(in_names)
    all_names = list(in_names) + out_names + ([part_name] if part_name else [])

    def _body(*args):
        operands = list(args)
        if part_name is not None:
            operands.append(bass2jax.partition_id_tensor())
        return tuple(
            bass2jax._bass_exec_p.bind(
                *operands,
                out_avals=tuple(out_avals),
                in_names=tuple(all_names),
                out_names=tuple(out_names),
                lowering_input_output_aliases=(),
                sim_require_finite=True,
                sim_require_nnan=True,
                nc=nc,
            )
        )

    n_outs = len(out_names)
    donate = tuple(range(n_params, n_params + n_outs))
    sharded = jax.jit(
        shard_map(
            _body, mesh=mesh,
            in_specs=(PartitionSpec("core"),) * (n_params + n_outs),
            out_specs=(PartitionSpec("core"),) * n_outs,
            check_rep=False,
        ),
        donate_argnums=donate,
        keep_unused=True,
    )

    zero_shapes = tuple(
        (N_CORES * av.shape[0], *av.shape[1:]) for av in out_avals
    )
    zero_dtypes = tuple(av.dtype for av in out_avals)
    zeros_fn = jax.jit(
        lambda: tuple(jnp.zeros(s, d) for s, d in zip(zero_shapes, zero_dtypes)),
        out_shardings=tuple(shard for _ in out_avals),
    )

    handle = {
        "sharded": sharded,
        "in_names": in_names,
        "out_names": out_names,
        "zeros": zeros_fn,
    }
    _cache[key] = handle
    return handle


def _run(nc, feeds):
    import jax

    h = _exec_handle(nc)
    _, shard = _mesh_shard()
    args = []
    for nm in h["in_names"]:
        a = feeds[nm]
        if isinstance(a, np.ndarray):
            a = jax.device_put(a, shard)
        args.append(a)
    zs = h["zeros"]()
    outs = h["sharded"](*args, *zs)
    return dict(zip(h["out_names"], outs))


# --------------------------------------------------------------------------
# host-side routing / packing
# --------------------------------------------------------------------------

def _route(x, gate_w, bias):
    logits = x.astype(np.float64) @ gate_w.T.astype(np.float64)
    scores = 1.0 / (1.0 + np.exp(-logits)) + bias.astype(np.float64)
    topk_idx = np.argsort(-scores, axis=1, kind="stable")[:, :TOPK]
    topk_w = np.take_along_axis(scores, topk_idx, axis=1)
    topk_w = topk_w / (topk_w.sum(axis=1, keepdims=True) + 1e-20)
    tok, wgt = [], []
    for e in range(E):
        sel = topk_idx == e
        rows = np.nonzero(sel.any(axis=1))[0].astype(np.int32)
        tok.append(rows)
        wgt.append((topk_w[rows] * sel[rows]).sum(axis=1).astype(np.float32))
    return tok, wgt


def _bf16(a):
    """Fast fp32 -> bf16 cast (round-to-nearest-even) via integer ops."""
    a = np.ascontiguousarray(a, dtype=np.float32)
    v = a.view(np.uint32)
    r = ((v + 0x7FFF + ((v >> 16) & 1)) >> 16).astype(np.uint16)
    return r.view(BF16).reshape(a.shape)


def _pack_gu(Wg_e, Wu_e):
    """[I, H] x2 (bf16) -> [IK, 128, 2*HK*128] tiled for stationary loads."""
    g = Wg_e.reshape(IK, 128, HK, 128).transpose(0, 3, 2, 1)  # [i, p, k, c]
    u = Wu_e.reshape(IK, 128, HK, 128).transpose(0, 3, 2, 1)
    out = np.empty((IK, 128, 2, HK, 128), dtype=BF16)
    out[:, :, 0] = g
    out[:, :, 1] = u
    return out.reshape(IK, 128, 2 * HK * 128)


def _pack_d(Wd_e):
    """[H, I] (bf16) -> [8, 128, IK*512] tiled for moving-operand loads."""
    return np.ascontiguousarray(
        Wd_e.reshape(8, 512, IK, 128).transpose(0, 3, 2, 1)
    ).reshape(8, 128, IK * 512)


def _pack_xt(xg):
    """[Cs, H] (bf16) -> [128, HK, Cs] token-transposed batch."""
    return np.ascontiguousarray(xg.reshape(-1, HK, 128).transpose(2, 1, 0))


def _prep(hidden_states, gate_w, bias, Wg, Wu, Wd, Sg, Su, Sd):
    """All host-side packing. Returns (C0, C1, feeds dict of global arrays)."""
    x = np.ascontiguousarray(hidden_states, dtype=np.float32).reshape(-1, H)

    tok, wgt = _route(x, gate_w, bias)
    counts = np.array([len(t) for t in tok])
    order = np.argsort(-counts, kind="stable")
    slot0 = order[:N_CORES]            # big experts, core c gets slot0[c]
    slot1 = order[N_CORES:][::-1]      # small experts, core c gets slot1[c]
    C0 = max(int(np.ceil(counts[slot0].max() / 128) * 128), 128)
    C1 = max(int(np.ceil(counts[slot1].max() / 128) * 128), 128)

    x_bf = _bf16(x)
    Wg_bf = _bf16(Wg)
    Wu_bf = _bf16(Wu)
    Wd_bf = _bf16(Wd)
    Sg_bf = _bf16(Sg)
    Su_bf = _bf16(Su)
    Sd_bf = _bf16(Sd)

    def slot_arrays(e, C):
        n = counts[e]
        idxp = np.zeros(C, np.int32)
        idxp[:n] = tok[e]
        wp = np.zeros(C, np.float32)
        wp[:n] = wgt[e]
        xt = _pack_xt(x_bf[idxp])
        idx_col = np.ascontiguousarray(idxp.reshape(-1, 128).T)
        w_col = np.ascontiguousarray(wp.reshape(-1, 128).T)
        return xt, idx_col, w_col

    xt0 = np.empty((N_CORES * 128, HK, C0), BF16)
    xt1 = np.empty((N_CORES * 128, HK, C1), BF16)
    xts = np.empty((N_CORES * 128, HK, CS_SHARED), BF16)
    idx0 = np.empty((N_CORES * 128, C0 // 128), np.int32)
    idx1 = np.empty((N_CORES * 128, C1 // 128), np.int32)
    w0 = np.empty((N_CORES * 128, C0 // 128), np.float32)
    w1 = np.empty((N_CORES * 128, C1 // 128), np.float32)
    wgu = np.empty((2 * N_CORES, IK, 128, 2 * HK * 128), BF16)
    wd = np.empty((2 * N_CORES, 8, 128, IK * 512), BF16)

    for c in range(N_CORES):
        e0, e1 = int(slot0[c]), int(slot1[c])
        xt0[c * 128:(c + 1) * 128], idx0[c * 128:(c + 1) * 128], \
            w0[c * 128:(c + 1) * 128] = slot_arrays(e0, C0)
        xt1[c * 128:(c + 1) * 128], idx1[c * 128:(c + 1) * 128], \
            w1[c * 128:(c + 1) * 128] = slot_arrays(e1, C1)
        xts[c * 128:(c + 1) * 128] = _pack_xt(
            x_bf[c * CS_SHARED:(c + 1) * CS_SHARED])
        wgu[2 * c] = _pack_gu(Wg_bf[e0], Wu_bf[e0])
        wgu[2 * c + 1] = _pack_gu(Wg_bf[e1], Wu_bf[e1])
        wd[2 * c] = _pack_d(Wd_bf[e0])
        wd[2 * c + 1] = _pack_d(Wd_bf[e1])

    # shared-expert weights: replicated (avoid device collectives for them)
    sgu_full = np.ascontiguousarray(_pack_gu(Sg_bf, Su_bf))  # [16,128,8192]
    sd_full = _pack_d(Sd_bf)                                 # [8,128,8192]
    sgu_rep = np.broadcast_to(
        sgu_full[None], (N_CORES,) + sgu_full.shape
    ).reshape(N_CORES * IK, 128, 2 * HK * 128)
    sd_rep = np.broadcast_to(
        sd_full[None], (N_CORES,) + sd_full.shape
    ).reshape(N_CORES * 8, 128, IK * 512)

    feeds = {
        "xt0": xt0, "xt1": xt1, "xts": xts,
        "idx0": idx0, "idx1": idx1, "w0": w0, "w1": w1,
        "wgu": wgu, "wd": wd,
        "sgu": np.ascontiguousarray(sgu_rep),
        "sd": np.ascontiguousarray(sd_rep),
    }
    return C0, C1, feeds


def kernel(hidden_states, gate_w, bias, Wg, Wu, Wd, Sg, Su, Sd):
    orig_shape = hidden_states.shape
    C0, C1, feeds = _prep(hidden_states, gate_w, bias, Wg, Wu, Wd, Sg, Su, Sd)

    key = ("moe", C0, C1)
    nc = _cache.get(key) or _cache.setdefault(key, _build_moe(C0, C1))

    outs = _run(nc, feeds)
    out = np.asarray(outs["final"]).astype(np.float32, copy=False)

    _cache["last_feeds"] = feeds
    _cache["last_key"] = key
    return out.reshape(orig_shape)


# revision 32
# speedup vs baseline: 1.0382x; 1.0382x over previous
"""Mistral4-style MoE block on 8 Trainium2 NeuronCores — single merged program.

Strategy (expert-parallel, sparse compute):
  - Router (sigmoid gate + top-4, weight normalization) runs on host in
    float64: tiny compute, gives the exact token->expert dispatch lists.
  - Host pre-gathers each expert's token batch and pre-tiles ALL weights
    into the exact SBUF layouts the matmuls need (contiguous 1-2MB blocks,
    no on-device DMA transposes). Shared-expert weights ship replicated
    (no device AllGather: collectives contend with the weight-stream DMAs
    for the 16 SDMA engines).
  - Experts are paired big+small per core (sorted by token count), so slot
    capacities are C0+C1 ~= max+median instead of 2*max.
  - One program per core:
      * slot0/slot1 (routed): gated MLP on the pre-gathered token batch;
        the per-token combine weight is folded into the PSUM-drain copy's
        `scale` operand; outputs scatter-add (indirect DMA, bf16) into
        column-half partial tensors [T, 2048].
      * two ReduceScatters (one per half) fire after all routed
        scatter-adds COMPLETE (collectives freeze concurrent DMA drain on
        this stack, so they must never overlap in-flight scatters); they
        are spaced apart so the shared slot's weight-prefetch buffers can
        coast through each window.
      * shared expert (slot2, tokens [512c,512(c+1))) runs last,
        overlapping the ReduceScatters; its output stays out of the RS.
      * final[512,H] = rs + shared_y.
"""

import sys

if "/opt/trn_rl_repo" not in sys.path:
    sys.path.insert(0, "/opt/trn_rl_repo")

import numpy as np
import ml_dtypes

T, H, I, E, TOPK = 4096, 4096, 2048, 16, 4
N_CORES = 8
CS_SHARED = T // N_CORES  # 512 shared-expert tokens per core
HK = H // 128  # 32 contraction chunks for up/gate
IK = I // 128  # 16 contraction chunks for down-proj
BF16 = ml_dtypes.bfloat16

_cache = {}


def _csplits(c, step=512):
    return [(c0, min(step, c - c0)) for c0 in range(0, c, step)]


# --------------------------------------------------------------------------
# program builder
# --------------------------------------------------------------------------

def _build_moe(C0, C1, wgu_bufs=2, wd_bufs=2, psa_bufs=4, psb_bufs=4,
               stage_bufs=2, ot_bufs=10, xt_chunk=512, nrs=2,
               memset_gate_i=6, rs_gate_i=8):
    import concourse.mybir as mybir
    import concourse.tile as tile
    import concourse.bass as bass
    from concourse.tile import add_dep_helper
    from concourse import bacc

    nc = bacc.Bacc("TRN2", target_bir_lowering=False, debug=False)
    dt = mybir.dt

    QW = H // nrs            # quarter width (1024 for nrs=4)
    HPQ = QW // 512          # h-blocks per quarter

    # ---- per-core external inputs ----
    xt0_d = nc.dram_tensor("xt0", [128, HK, C0], dt.bfloat16, kind="ExternalInput")
    xt1_d = nc.dram_tensor("xt1", [128, HK, C1], dt.bfloat16, kind="ExternalInput")
    xts_d = nc.dram_tensor("xts", [128, HK, CS_SHARED], dt.bfloat16,
                           kind="ExternalInput")
    # up/gate weights, tiled: [slot, i_block, p, (gu, k, c)]
    wgu_d = nc.dram_tensor("wgu", [2, IK, 128, 2 * HK * 128], dt.bfloat16,
                           kind="ExternalInput")
    # down weights, tiled: [slot, h_block, p, (k, n)]
    wd_d = nc.dram_tensor("wd", [2, 8, 128, IK * 512], dt.bfloat16,
                          kind="ExternalInput")
    # shared-expert weights (full, replicated to every core)
    sgu_d = nc.dram_tensor("sgu", [IK, 128, 2 * HK * 128], dt.bfloat16,
                           kind="ExternalInput")
    sd_d = nc.dram_tensor("sd", [8, 128, IK * 512], dt.bfloat16,
                          kind="ExternalInput")
    idx0_d = nc.dram_tensor("idx0", [128, C0 // 128], dt.int32,
                            kind="ExternalInput")
    idx1_d = nc.dram_tensor("idx1", [128, C1 // 128], dt.int32,
                            kind="ExternalInput")
    w0_d = nc.dram_tensor("w0", [128, C0 // 128], dt.float32,
                          kind="ExternalInput")
    w1_d = nc.dram_tensor("w1", [128, C1 // 128], dt.float32,
                          kind="ExternalInput")
    final_d = nc.dram_tensor("final", [CS_SHARED, H], dt.float32,
                             kind="ExternalOutput")

    # ---- internal DRAM ----
    partial_q = [nc.dram_tensor(f"partial{q}", [T, QW], dt.bfloat16)
                 for q in range(nrs)]
    rs_q = [nc.dram_tensor(f"rs{q}", [CS_SHARED, QW], dt.bfloat16)
            for q in range(nrs)]
    ys_dram = nc.dram_tensor("ys", [CS_SHARED, H], dt.bfloat16)

    with tile.TileContext(nc) as tc:
        with (
            tc.tile_pool(name="xT", bufs=1) as xT_pool,
            tc.tile_pool(name="zT", bufs=1) as zT_pool,
            tc.tile_pool(name="wgu", bufs=wgu_bufs) as wgu_pool,
            tc.tile_pool(name="wd", bufs=wd_bufs) as wd_pool,
            tc.tile_pool(name="stage", bufs=stage_bufs) as stage_pool,
            tc.tile_pool(name="ot", bufs=ot_bufs) as ot_pool,
            tc.tile_pool(name="wcol", bufs=2) as wcol_pool,
            tc.tile_pool(name="idx", bufs=2) as idx_pool,
            tc.tile_pool(name="zero", bufs=1) as zero_pool,
            tc.tile_pool(name="fin", bufs=2) as fin_pool,
            tc.tile_pool(name="psA", bufs=psa_bufs, space="PSUM") as psum_a,
            tc.tile_pool(name="psB", bufs=psb_bufs, space="PSUM") as psum_b,
        ):
            slots = [
                (xt0_d, C0, 0, idx0_d, w0_d),
                (xt1_d, C1, 1, idx1_d, w1_d),
                (xts_d, CS_SHARED, None, None, None),
            ]

            rs_insts = []
            for s, (xt_ap, Cs, wslot, idx_ap, w_ap) in enumerate(slots):
                ct = Cs // 128

                # load xT in k-plane chunks: contiguous per-partition runs
                # (one descriptor per partition) instead of 1KB fragments
                xT = xT_pool.tile([128, HK, Cs], dt.bfloat16, tag="xT")
                for k0 in range(0, HK, 8):
                    nc.scalar.dma_start(out=xT[:, k0:k0 + 8, :],
                                        in_=xt_ap[:, k0:k0 + 8, :])

                if idx_ap is not None:
                    it = idx_pool.tile([128, ct], dt.int32, tag="idx")
                    nc.sync.dma_start(out=it[:], in_=idx_ap[:, :ct])
                    wc = wcol_pool.tile([128, ct], dt.float32, tag="wcol")
                    nc.sync.dma_start(out=wc[:], in_=w_ap[:, :ct])

                # ---- up/gate -> zT ----
                zT = zT_pool.tile([128, IK, Cs], dt.bfloat16, tag="zT")
                for i in range(IK):
                    wgu_i = wgu_pool.tile([128, 2, HK, 128], dt.bfloat16,
                                          tag="wgu")
                    src = wgu_d[wslot, i] if wslot is not None else sgu_d[i]
                    wgu_load = nc.sync.dma_start(
                        out=wgu_i[:],
                        in_=src.rearrange("p (g k c) -> p g k c", g=2, k=HK),
                    )
                    # space the second ReduceScatter into mid-shared-slot so
                    # the weight-prefetch buffers can coast through each RS
                    # window separately
                    if s == 2 and i == rs_gate_i and len(rs_insts) < nrs:
                        q = len(rs_insts)
                        cc = nc.gpsimd.collective_compute(
                            "ReduceScatter", mybir.AluOpType.add,
                            replica_groups=[list(range(N_CORES))],
                            ins=[partial_q[q][:]], outs=[rs_q[q][:]],
                        )
                        add_dep_helper(cc.ins, wgu_load.ins,
                                       reason="space RS into shared slot")
                        rs_insts.append(cc)
                    for c0, cw in _csplits(Cs):
                        pg = psum_a.tile([128, cw], dt.float32, tag="psA")
                        pu = psum_a.tile([128, cw], dt.float32, tag="psA")
                        for k in range(HK):
                            nc.tensor.matmul(
                                pg[:], wgu_i[:, 0, k, :], xT[:, k, c0:c0 + cw],
                                start=(k == 0), stop=(k == HK - 1),
                            )
                        for k in range(HK):
                            nc.tensor.matmul(
                                pu[:], wgu_i[:, 1, k, :], xT[:, k, c0:c0 + cw],
                                start=(k == 0), stop=(k == HK - 1),
                            )
                        g_s = stage_pool.tile([128, cw], dt.float32, tag="stage")
                        nc.scalar.activation(
                            g_s[:], pg[:], mybir.ActivationFunctionType.Silu
                        )
                        nc.vector.tensor_mul(zT[:, i, c0:c0 + cw], g_s[:], pu[:])

                    # zero-init the partial quarters off the critical head,
                    # hidden under slot0's up/gate
                    if s == 0 and i == memset_gate_i:
                        # hold the memset off the critical head: the idle
                        # gpsimd queue would otherwise flood HBM at t=0 and
                        # starve the weight/xt loads the PE waits on. The
                        # 1-element rewrite below adds a real data dep on this
                        # i-iteration's silu output, so the stores can't be
                        # scheduled before mid-slot0.
                        zb = 4096 // QW if QW < 4096 else 1
                        zt = zero_pool.tile([128, zb * QW], dt.bfloat16)
                        nc.vector.memset(zt[:], 0.0)
                        nc.vector.tensor_scalar_mul(
                            zt[0:1, 0:1], g_s[0:1, 0:1], 0.0)
                        for q in range(nrs):
                            view = partial_q[q].rearrange(
                                "(a b) c -> a (b c)", b=zb)
                            for tb in range(T // zb // 128):
                                nc.gpsimd.dma_start(
                                    out=view[tb * 128:(tb + 1) * 128, :],
                                    in_=zt[:])

                # ---- down-proj -> scatter-add / ys ----
                for h in range(8):
                    wd_h = wd_pool.tile([128, IK, 512], dt.bfloat16, tag="wd")
                    src = wd_d[wslot, h] if wslot is not None else sd_d[h]
                    nc.sync.dma_start(
                        out=wd_h[:],
                        in_=src.rearrange("p (k n) -> p k n", k=IK),
                    )
                    for cb in range(ct):
                        po = psum_b.tile([128, 512], dt.float32, tag="psB")
                        for k in range(IK):
                            nc.tensor.matmul(
                                po[:], zT[:, k, cb * 128:(cb + 1) * 128],
                                wd_h[:, k, :],
                                start=(k == 0), stop=(k == IK - 1),
                            )
                        ot = ot_pool.tile([128, 512], dt.bfloat16, tag="ot")
                        if idx_ap is not None:
                            nc.scalar.activation(
                                ot[:], po[:], mybir.ActivationFunctionType.Copy,
                                scale=wc[:, cb:cb + 1],
                            )
                            nc.gpsimd.indirect_dma_start(
                                out=partial_q[h // HPQ][:],
                                out_offset=bass.IndirectOffsetOnAxis(
                                    ap=it[:, cb:cb + 1], axis=0),
                                in_=ot[:],
                                in_offset=None,
                                element_offset=(h % HPQ) * 512,
                                compute_op=mybir.AluOpType.add,
                            )
                        else:
                            nc.scalar.activation(
                                ot[:], po[:], mybir.ActivationFunctionType.Copy
                            )
                            nc.scalar.dma_start(
                                out=ys_dram[cb * 128:(cb + 1) * 128,
                                            h * 512:(h + 1) * 512],
                                in_=ot[:],
                            )

                # first ReduceScatter fires once all routed scatter-adds have
                # COMPLETED (a collective freezes concurrent DMA drain, so an
                # RS that starts while scatters are still in flight deadlocks
                # the drain pipeline until the RS finishes). The probe reads
                # force completion-ordering: their dispatch waits on every
                # partial writer's DMA-completion semaphore, and the RS sits
                # behind them on the in-order gpsimd queue.
                if s == 1:
                    for q in range(nrs):
                        probe = fin_pool.tile([128, 16], dt.bfloat16,
                                              tag="rsb")
                        nc.gpsimd.dma_start(out=probe[:],
                                            in_=partial_q[q][0:128, 0:16])
                    cc = nc.gpsimd.collective_compute(
                        "ReduceScatter", mybir.AluOpType.add,
                        replica_groups=[list(range(N_CORES))],
                        ins=[partial_q[0][:]], outs=[rs_q[0][:]],
                    )
                    rs_insts.append(cc)

            # ---- final = rs + ys ----
            # q-major: the q=0 half only needs RS_lo + the shared expert's
            # h<4 down-proj outputs, so it completes while RS_hi / the rest
            # of the shared slot still run. rs loads ride the sync ring (the
            # gpsimd queue would serialize them behind the second RS).
            FW = 512
            for f0 in range(0, H, FW):
                q, qo = f0 // QW, f0 % QW
                for rb in range(CS_SHARED // 128):
                    rsb = fin_pool.tile([128, FW], dt.bfloat16, tag="rsb")
                    nc.sync.dma_start(
                        out=rsb[:],
                        in_=rs_q[q][rb * 128:(rb + 1) * 128, qo:qo + FW])
                    ysb = fin_pool.tile([128, FW], dt.bfloat16, tag="ysb")
                    nc.scalar.dma_start(
                        out=ysb[:],
                        in_=ys_dram[rb * 128:(rb + 1) * 128, f0:f0 + FW])
                    ob = fin_pool.tile([128, FW], dt.float32, tag="ob")
                    nc.vector.tensor_add(ob[:], rsb[:], ysb[:])
                    nc.sync.dma_start(
                        out=final_d[rb * 128:(rb + 1) * 128, f0:f0 + FW],
                        in_=ob[:])

    nc.compile()
    return nc


# --------------------------------------------------------------------------
# execution plumbing (cached jitted SPMD launch)
# --------------------------------------------------------------------------

def _mesh_shard():
    import jax
    from jax.sharding import Mesh, PartitionSpec, NamedSharding

    if "mesh" not in _cache:
        devices = jax.devices()[:N_CORES]
        mesh = Mesh(np.asarray(devices), ("core",))
        _cache["mesh"] = mesh
        _cache["shard"] = NamedSharding(mesh, PartitionSpec("core"))
    return _cache["mesh"], _cache["shard"]


def _exec_handle(nc):
    import jax
    import jax.numpy as jnp
    from jax.sharding import PartitionSpec
    from jax.experimental.shard_map import shard_map
    import concourse.mybir as mybir
    from concourse import bass2jax

    key = id(nc)
    if key in _cache:
        return _cache[key]

    bass2jax.install_neuronx_cc_hook()
    mesh, shard = _mesh_shard()

    part_name = nc.partition_id_tensor.name if nc.partition_id_tensor else None
    in_names, out_names, out_avals = [], [], []
    for alloc in nc.m.functions[0].allocations:
        if not isinstance(alloc, mybir.MemoryLocationSet):
            continue
        name = alloc.memorylocations[0].name
        if alloc.kind == "ExternalInput":
            if name != part_name:
                in_names.append(name)
        elif alloc.kind == "ExternalOutput":
            out_names.append(name)
            out_avals.append(
                jax.core.ShapedArray(tuple(alloc.tensor_shape),
                                     mybir.dt.np(alloc.dtype))
            )
    n_params = len(in_names)
    all_names = list(in_names) + out_names + ([part_name] if part_name else [])

    def _body(*args):
        operands = list(args)
        if part_name is not None:
            operands.append(bass2jax.partition_id_tensor())
        return tuple(
            bass2jax._bass_exec_p.bind(
                *operands,
                out_avals=tuple(out_avals),
                in_names=tuple(all_names),
                out_names=tuple(out_names),
                lowering_input_output_aliases=(),
                sim_require_finite=True,
                sim_require_nnan=True,
                nc=nc,
            )
        )

    n_outs = len(out_names)
    donate = tuple(range(n_params, n_params + n_outs))
    sharded = jax.jit(
        shard_map(
            _body, mesh=mesh,
            in_specs=(PartitionSpec("core"),) * (n_params + n_outs),
            out_specs=(PartitionSpec("core"),) * n_outs,
            check_rep=False,
        ),
        donate_argnums=donate,
        keep_unused=True,
    )

    zero_shapes = tuple(
        (N_CORES * av.shape[0], *av.shape[1:]) for av in out_avals
    )
    zero_dtypes = tuple(av.dtype for av in out_avals)
    zeros_fn = jax.jit(
        lambda: tuple(jnp.zeros(s, d) for s, d in zip(zero_shapes, zero_dtypes)),
        out_shardings=tuple(shard for _ in out_avals),
    )

    handle = {
        "sharded": sharded,
        "in_names": in_names,
        "out_names": out_names,
        "zeros": zeros_fn,
    }
    _cache[key] = handle
    return handle


def _run(nc, feeds):
    import jax

    h = _exec_handle(nc)
    _, shard = _mesh_shard()
    args = []
    for nm in h["in_names"]:
        a = feeds[nm]
        if isinstance(a, np.ndarray):
            a = jax.device_put(a, shard)
        args.append(a)
    zs = h["zeros"]()
    outs = h["sharded"](*args, *zs)
    return dict(zip(h["out_names"], outs))


# --------------------------------------------------------------------------
# host-side routing / packing
# --------------------------------------------------------------------------

def _route(x, gate_w, bias):
    logits = x.astype(np.float64) @ gate_w.T.astype(np.float64)
    scores = 1.0 / (1.0 + np.exp(-logits)) + bias.astype(np.float64)
    topk_idx = np.argsort(-scores, axis=1, kind="stable")[:, :TOPK]
    topk_w = np.take_along_axis(scores, topk_idx, axis=1)
    topk_w = topk_w / (topk_w.sum(axis=1, keepdims=True) + 1e-20)
    tok, wgt = [], []
    for e in range(E):
        sel = topk_idx == e
        rows = np.nonzero(sel.any(axis=1))[0].astype(np.int32)
        tok.append(rows)
        wgt.append((topk_w[rows] * sel[rows]).sum(axis=1).astype(np.float32))
    return tok, wgt


def _bf16(a):
    """Fast fp32 -> bf16 cast (round-to-nearest-even) via integer ops."""
    a = np.ascontiguousarray(a, dtype=np.float32)
    v = a.view(np.uint32)
    r = ((v + 0x7FFF + ((v >> 16) & 1)) >> 16).astype(np.uint16)
    return r.view(BF16).reshape(a.shape)


def _pack_gu(Wg_e, Wu_e):
    """[I, H] x2 (bf16) -> [IK, 128, 2*HK*128] tiled for stationary loads."""
    g = Wg_e.reshape(IK, 128, HK, 128).transpose(0, 3, 2, 1)  # [i, p, k, c]
    u = Wu_e.reshape(IK, 128, HK, 128).transpose(0, 3, 2, 1)
    out = np.empty((IK, 128, 2, HK, 128), dtype=BF16)
    out[:, :, 0] = g
    out[:, :, 1] = u
    return out.reshape(IK, 128, 2 * HK * 128)


def _pack_d(Wd_e):
    """[H, I] (bf16) -> [8, 128, IK*512] tiled for moving-operand loads."""
    return np.ascontiguousarray(
        Wd_e.reshape(8, 512, IK, 128).transpose(0, 3, 2, 1)
    ).reshape(8, 128, IK * 512)


def _pack_xt(xg):
    """[Cs, H] (bf16) -> [128, HK, Cs] token-transposed batch."""
    return np.ascontiguousarray(xg.reshape(-1, HK, 128).transpose(2, 1, 0))


def _prep(hidden_states, gate_w, bias, Wg, Wu, Wd, Sg, Su, Sd):
    """All host-side packing. Returns (C0, C1, feeds dict of global arrays)."""
    x = np.ascontiguousarray(hidden_states, dtype=np.float32).reshape(-1, H)

    tok, wgt = _route(x, gate_w, bias)
    counts = np.array([len(t) for t in tok])
    order = np.argsort(-counts, kind="stable")
    slot0 = order[:N_CORES]            # big experts, core c gets slot0[c]
    slot1 = order[N_CORES:][::-1]      # small experts, core c gets slot1[c]
    C0 = max(int(np.ceil(counts[slot0].max() / 128) * 128), 128)
    C1 = max(int(np.ceil(counts[slot1].max() / 128) * 128), 128)

    x_bf = _bf16(x)
    Wg_bf = _bf16(Wg)
    Wu_bf = _bf16(Wu)
    Wd_bf = _bf16(Wd)
    Sg_bf = _bf16(Sg)
    Su_bf = _bf16(Su)
    Sd_bf = _bf16(Sd)

    def slot_arrays(e, C):
        n = counts[e]
        idxp = np.zeros(C, np.int32)
        idxp[:n] = tok[e]
        wp = np.zeros(C, np.float32)
        wp[:n] = wgt[e]
        xt = _pack_xt(x_bf[idxp])
        idx_col = np.ascontiguousarray(idxp.reshape(-1, 128).T)
        w_col = np.ascontiguousarray(wp.reshape(-1, 128).T)
        return xt, idx_col, w_col

    xt0 = np.empty((N_CORES * 128, HK, C0), BF16)
    xt1 = np.empty((N_CORES * 128, HK, C1), BF16)
    xts = np.empty((N_CORES * 128, HK, CS_SHARED), BF16)
    idx0 = np.empty((N_CORES * 128, C0 // 128), np.int32)
    idx1 = np.empty((N_CORES * 128, C1 // 128), np.int32)
    w0 = np.empty((N_CORES * 128, C0 // 128), np.float32)
    w1 = np.empty((N_CORES * 128, C1 // 128), np.float32)
    wgu = np.empty((2 * N_CORES, IK, 128, 2 * HK * 128), BF16)
    wd = np.empty((2 * N_CORES, 8, 128, IK * 512), BF16)

    for c in range(N_CORES):
        e0, e1 = int(slot0[c]), int(slot1[c])
        xt0[c * 128:(c + 1) * 128], idx0[c * 128:(c + 1) * 128], \
            w0[c * 128:(c + 1) * 128] = slot_arrays(e0, C0)
        xt1[c * 128:(c + 1) * 128], idx1[c * 128:(c + 1) * 128], \
            w1[c * 128:(c + 1) * 128] = slot_arrays(e1, C1)
        xts[c * 128:(c + 1) * 128] = _pack_xt(
            x_bf[c * CS_SHARED:(c + 1) * CS_SHARED])
        wgu[2 * c] = _pack_gu(Wg_bf[e0], Wu_bf[e0])
        wgu[2 * c + 1] = _pack_gu(Wg_bf[e1], Wu_bf[e1])
        wd[2 * c] = _pack_d(Wd_bf[e0])
        wd[2 * c + 1] = _pack_d(Wd_bf[e1])

    # shared-expert weights: replicated (avoid device collectives for them)
    sgu_full = np.ascontiguousarray(_pack_gu(Sg_bf, Su_bf))  # [16,128,8192]
    sd_full = _pack_d(Sd_bf)                                 # [8,128,8192]
    sgu_rep = np.broadcast_to(
        sgu_full[None], (N_CORES,) + sgu_full.shape
    ).reshape(N_CORES * IK, 128, 2 * HK * 128)
    sd_rep = np.broadcast_to(
        sd_full[None], (N_CORES,) + sd_full.shape
    ).reshape(N_CORES * 8, 128, IK * 512)

    feeds = {
        "xt0": xt0, "xt1": xt1, "xts": xts,
        "idx0": idx0, "idx1": idx1, "w0": w0, "w1": w1,
        "wgu": wgu, "wd": wd,
        "sgu": np.ascontiguousarray(sgu_rep),
        "sd": np.ascontiguousarray(sd_rep),
    }
    return C0, C1, feeds


def kernel(hidden_states, gate_w, bias, Wg, Wu, Wd, Sg, Su, Sd):
    orig_shape = hidden_states.shape
    C0, C1, feeds = _prep(hidden_states, gate_w, bias, Wg, Wu, Wd, Sg, Su, Sd)

    key = ("moe", C0, C1)
    nc = _cache.get(key) or _cache.setdefault(key, _build_moe(C0, C1))

    outs = _run(nc, feeds)
    out = np.asarray(outs["final"]).astype(np.float32, copy=False)

    _cache["last_feeds"] = feeds
    _cache["last_key"] = key
    return out.reshape(orig_shape)
